# revision 2
# baseline (speedup 1.0000x reference)
"""Trainium2 Bass kernel for 2-layer GCN (GCNConv -> ReLU -> GCNConv).

v2 strategy — SBUF-resident fp16 tables + transpose-mode SBUF-source gathers
(the baseline's HBM dma_gather was HBM-latency bound at ~63ns/edge):

- Both layers reduce to: gather 16-wide rows t[src], segment-sum by dst
  (linear layers commute with the normalized aggregation).
- The 16-fp16 (32B) node records live in SBUF as [128 part, W windows] of
  256B chunks; chunk (tok, w) holds 8 records at positions s=0..7.
- A token (edge) gathers its source's 256B chunk via dma_gather(transpose=
  True, SBUF source): the chunk becomes a 128-partition fp16 column; the
  wanted record sits at partition slice [16s, 16s+16) where s = the record's
  chunk position ("class").  Chunk-mates land on other slices — never read.
- Host assigns each node TWO candidate classes and each edge picks one
  (power-of-two-choices), balancing per-(destination-group, class) slot
  counts K.  Grid columns per group g: [class s][slot k<K[g,s]][win j][dst p]
  so one strided DVE tensor_reduce per (group, class) segment-sums slot
  layers into slice s of a [128, 256] tile P.  P's 8 slices collapse via a
  PE matmul with a 0/1 selector (layer 2 fuses W2 into the selector).
- 3 SPMD launches: A (t1 = dinv*x@W1), B (layer-1 aggregate -> t2),
  C (layer-2 aggregate -> @W2+b2).  Host re-shards tables between launches.
"""
import os
import sys

sys.path.insert(0, "/opt/trn_rl_repo")

import numpy as np

import concourse.bass as bass
import concourse.mybir as mybir
import concourse.tile as tile
from concourse import bacc, bass_utils, library_config

N = 100000
E = 1600000
DIN, HID, DOUT = 256, 16, 64
NDEV = 8
NCLS = 8
GW = 2                      # windows per K-uniform group
F32 = mybir.dt.float32
F16 = mybir.dt.float16
I16 = mybir.dt.int16
NQUEUES = 1

LAST_EXEC_NS = []


# ----------------------------------------------------------------------------
# host-side graph planning
# ----------------------------------------------------------------------------

def _ragged_arange(lens):
    ends = np.cumsum(lens)
    total = int(ends[-1]) if len(lens) else 0
    out = np.arange(total, dtype=np.int64)
    out -= np.repeat(ends - lens, lens)
    return out


def _plan(edge_index):
    rng = np.random.default_rng(12345)
    src = np.asarray(edge_index[0], dtype=np.int64)
    dst = np.asarray(edge_index[1], dtype=np.int64)
    all_src = np.concatenate([src, np.arange(N, dtype=np.int64)])
    all_dst = np.concatenate([dst, np.arange(N, dtype=np.int64)])
    T = len(all_src)
    indeg = np.bincount(all_dst, minlength=N).astype(np.int64)
    # all_dst already includes the self-loop, so indeg IS the GCN degree
    dinv_n = (1.0 / np.sqrt(indeg.astype(np.float64))).astype(np.float32)

    # rank deal: degree-sorted; i-th -> device i%8, window (i//8)//128
    order = np.argsort(-indeg, kind="stable")
    di = np.empty(N, np.int64)
    di[order] = np.arange(N)
    dev_n = di % NDEV
    w_n = (di // NDEV) // 128
    p_n = (di // NDEV) % 128
    nwin = int(w_n.max()) + 1
    ngrp = (nwin + GW - 1) // GW
    nwin_pad = ngrp * GW
    rank_n = (w_n * NDEV + dev_n) * 128 + p_n
    nloc = nwin_pad * 128
    npad = nloc * NDEV

    grp_n = w_n // GW

    # --- 2-choice class assignment ---
    s1 = rng.integers(0, NCLS, N)
    s2 = (s1 + 1 + rng.integers(0, NCLS - 1, N)) % NCLS
    ko = np.argsort(all_dst, kind="stable")
    t_dst = all_dst[ko]
    t_src = all_src[ko]
    starts = np.searchsorted(t_dst, np.arange(N + 1))
    pos = np.arange(T) - np.repeat(starts[:-1], np.diff(starts))
    cnt = np.zeros((N, NCLS), np.int16)
    cls_tok = np.zeros(T, np.int8)
    for k in range(int(indeg.max()) + 1):
        m = np.flatnonzero(pos == k)
        if len(m) == 0:
            break
        u, v = t_src[m], t_dst[m]
        c1, c2 = s1[u], s2[u]
        c = np.where(cnt[v, c1] <= cnt[v, c2], c1, c2)
        cls_tok[m] = c
        cnt[v, c] += 1

    def _getK(ct):
        K = np.zeros((ngrp, NCLS), np.int64)
        np.maximum.at(K, (grp_n[t_dst], ct.astype(np.int64)), cnt[t_dst, ct])
        return K

    K = _getK(cls_tok)
    best, best_cls = K.sum(), cls_tok.copy()
    tc1, tc2 = s1[t_src], s2[t_src]
    for _ in range(8):
        cur = cls_tok.astype(np.int64)
        alt = np.where(cur == tc1, tc2, tc1)
        v = t_dst
        b = grp_n[v]
        cand = np.flatnonzero((cnt[v, cur] == K[b, cur])
                              & (cnt[v, alt] + 1 < K[b, alt]) & (cur != alt))
        if len(cand) == 0:
            break
        key = v[cand] * NCLS + cur[cand]
        ks = np.argsort(key, kind="stable")
        kk = key[ks]
        first = np.concatenate([[True], kk[1:] != kk[:-1]])
        mv = cand[ks[first]]
        cv, av, vv = cur[mv], alt[mv], v[mv]
        cls_tok[mv] = av.astype(np.int8)
        np.subtract.at(cnt, (vv, cv), 1)
        np.add.at(cnt, (vv, av), 1)
        K = _getK(cls_tok)
        if K.sum() < best:
            best, best_cls = K.sum(), cls_tok.copy()
    if K.sum() != best:
        cls_tok = best_cls
        cnt = np.zeros((N, NCLS), np.int16)
        np.add.at(cnt, (t_dst, cls_tok.astype(np.int64)), 1)
        K = _getK(cls_tok)
    K = np.maximum(K, 1)

    # --- table slot allocation per class ---
    used = np.zeros((N, NCLS), bool)
    used[t_src, cls_tok.astype(np.int64)] = True
    gidx = np.zeros((N, NCLS), np.int32)
    slot_u, slot_s, slot_q = [], [], []
    nwt = 0
    for s in range(NCLS):
        us = np.flatnonzero(used[:, s])
        q = np.arange(len(us))
        gidx[us, s] = (q // 128) * 128 + (q % 128)
        slot_u.append(us)
        slot_s.append(np.full(len(us), s))
        slot_q.append(q)
        nwt = max(nwt, (len(us) + 127) // 128)
    slot_u = np.concatenate(slot_u)
    slot_s = np.concatenate(slot_s)
    slot_q = np.concatenate(slot_q)
    idx_pad = nwt * 128                    # zero window
    assert idx_pad + 127 < 32768

    # --- grid column layout (global K; identical on all devices) ---
    offs = np.concatenate([np.zeros((ngrp, 1), np.int64),
                           np.cumsum(K, axis=1)], axis=1) * (GW * 128)
    Cg = offs[:, -1]
    grpbase = np.concatenate([[0], np.cumsum(Cg)])
    T_dev = int(grpbase[-1])
    assert T_dev % 16 == 0

    # gather split per group: class boundary nearest the middle
    split = []
    for g in range(ngrp):
        sh = int(np.argmin(np.abs(offs[g, 1:-1] - Cg[g] / 2))) + 1
        split.append((sh, int(offs[g, sh])))

    # --- token -> column, idx arrays per device ---
    occ = np.empty(T, np.int64)
    key2 = t_dst * NCLS + cls_tok.astype(np.int64)
    k2o = np.argsort(key2, kind="stable")
    kk2 = key2[k2o]
    bnd = np.concatenate([[True], kk2[1:] != kk2[:-1]])
    gstarts = np.flatnonzero(bnd)
    glens = np.diff(np.concatenate([gstarts, [T]]))
    occ[k2o] = _ragged_arange(glens)

    v = t_dst
    g = grp_n[v]
    col = (grpbase[g] + offs[g, cls_tok.astype(np.int64)]
           + occ * (GW * 128) + (w_n[v] % GW) * 128 + p_n[v])
    tdev = dev_n[v]
    srcval = gidx[t_src, cls_tok.astype(np.int64)].astype(np.int16)
    idxw = np.empty((NDEV, 128, T_dev // 16), np.int16)
    for d in range(NDEV):
        m = tdev == d
        a = np.full(T_dev, idx_pad, np.int16)
        a[col[m]] = srcval[m]
        idxw[d] = np.tile(a.reshape(T_dev // 16, 16).T, (8, 1))

    # --- per-device aux arrays ---
    ridx = np.empty((NDEV, nloc), np.int64)
    for d in range(NDEV):
        gg = ((np.arange(nwin_pad) * NDEV + d)[:, None] * 128 + np.arange(128))
        ridx[d] = gg.reshape(-1)
    node_of_rank = np.full(npad, -1, np.int64)
    node_of_rank[rank_n] = np.arange(N)
    dinv_r = np.zeros(npad, np.float32)
    dinv_r[rank_n] = dinv_n

    dinva = np.empty((NDEV, 128, nwin_pad), np.float32)
    dinvw = np.empty((NDEV, 64, nloc), np.float16)
    for d in range(NDEV):
        dr = dinv_r[ridx[d]]
        dinva[d] = dr.reshape(nwin_pad, 128).T
        dinvw[d] = np.tile(dr[None, :], (64, 1)).astype(np.float16)

    return dict(
        nwin=nwin_pad, ngrp=ngrp, nloc=nloc, npad=npad, nwt=nwt,
        idx_pad=idx_pad, K=K, offs=offs, Cg=Cg, grpbase=grpbase,
        T_dev=T_dev, split=split, idxw=idxw, ridx=ridx,
        node_of_rank=node_of_rank, rank_n=rank_n, dinva=dinva, dinvw=dinvw,
        slot_u=slot_u, slot_s=slot_s, slot_q=slot_q,
    )


# ----------------------------------------------------------------------------
# device programs
# ----------------------------------------------------------------------------

def _build_A(plan):
    nwin, nloc = plan["nwin"], plan["nloc"]
    nc = bacc.Bacc("TRN2", target_bir_lowering=False, debug=False,
                   num_devices=NDEV)
    xT_d = nc.dram_tensor("xT", [DIN, nloc], F32, kind="ExternalInput").ap()
    dinva_d = nc.dram_tensor("dinva", [128, nwin], F32,
                             kind="ExternalInput").ap()
    w1_d = nc.dram_tensor("w1", [128, 2, HID], F32, kind="ExternalInput").ap()
    t1_d = nc.dram_tensor("t1", [nloc, HID], F16, kind="ExternalOutput").ap()

    with tile.TileContext(nc) as tc:
        with (
            tc.tile_pool(name="cst", bufs=1) as cst,
            tc.tile_pool(name="xp", bufs=3) as xp,
            tc.tile_pool(name="ps", bufs=2, space="PSUM") as psp,
            tc.tile_pool(name="stg", bufs=2) as stg,
        ):
            w1t = cst.tile([128, 2, HID], F32)
            nc.sync.dma_start(out=w1t[:], in_=w1_d[:])
            dat = cst.tile([128, nwin], F32)
            nc.sync.dma_start(out=dat[:], in_=dinva_d[:])
            ov = t1_d.rearrange("(w p) f -> p w f", p=128)
            for i0 in range(0, nwin, 8):
                nb = min(8, nwin - i0)
                xts = []
                for k in range(2):
                    xt = xp.tile([128, 8 * 128], F32, tag=f"xt{k}")
                    nc.sync.dma_start(
                        out=xt[:, :nb * 128],
                        in_=xT_d[k * 128:(k + 1) * 128,
                                 i0 * 128:(i0 + nb) * 128],
                    )
                    xts.append(xt)
                stage = stg.tile([128, 8, HID], F16)
                for ib in range(nb):
                    ps = psp.tile([128, HID], F32)
                    for k in range(2):
                        nc.tensor.matmul(
                            out=ps[:],
                            lhsT=xts[k][:, ib * 128:(ib + 1) * 128],
                            rhs=w1t[:, k, :],
                            start=(k == 0), stop=(k == 1),
                        )
                    nc.vector.tensor_scalar_mul(
                        out=stage[:, ib, :], in0=ps[:],
                        scalar1=dat[:, i0 + ib:i0 + ib + 1],
                    )
                nc.sync.dma_start(out=ov[:, i0:i0 + nb, :],
                                  in_=stage[:, :nb, :])
    nc.compile()
    return nc


def _sbuf_gather(nc, out_sl, tabt, idx_sl, ncols, q):
    nc.gpsimd.dma_gather(
        out_sl, tabt[:], idx_sl,
        num_idxs=ncols, num_idxs_reg=ncols, elem_size=128,
        transpose=True, single_packet=False, queue_num=q,
        sbuf_tokens_per_rank=128, sbuf_free_dim_per_rank=256,
        sbuf_free_dim_pad_per_rank=0, sbuf_byte_offset=0,
    )


def _reduce_cls(nc, Ps, vt, c0, Kgs):
    """Full-128-partition strided segment-sum of one class's token columns.
    Only partitions [16s, 16s+16) of the result are meaningful; the selector
    matmul extracts them (DVE cost is free-size driven, partitions are
    parallel lanes, and 16-partition slices at odd 16-offsets violate the
    32-alignment rule)."""
    sl = vt[:, 0, c0:c0 + GW * 128 * Kgs]
    rap = bass.AP(
        sl.tensor, sl.offset,
        [list(sl.ap[0]), [1, GW * 128], [GW * 128, Kgs]],
    )
    nc.vector.tensor_reduce(
        out=Ps[:], in_=rap,
        axis=mybir.AxisListType.X, op=mybir.AluOpType.add,
    )


def _build_BC(plan, layer):
    nwin, ngrp, nloc, nwt = (plan["nwin"], plan["ngrp"], plan["nloc"],
                             plan["nwt"])
    K, offs, Cg, grpbase, split, T_dev = (plan["K"], plan["offs"], plan["Cg"],
                                          plan["grpbase"], plan["split"],
                                          plan["T_dev"])
    WB = GW * 128
    cg_max = max(int(c) for c in Cg)

    nc = bacc.Bacc("TRN2", target_bir_lowering=False, debug=False,
                   num_devices=NDEV, num_swdge_queues=NQUEUES)
    tab_d = nc.dram_tensor("tab", [128, (nwt + 1) * 128], F16,
                           kind="ExternalInput").ap()
    idx_d = nc.dram_tensor("idx", [128, T_dev // 16], I16,
                           kind="ExternalInput").ap()
    dinvw_d = nc.dram_tensor("dinvw", [64, nloc], F16,
                             kind="ExternalInput").ap()
    if layer == 1:
        b_d = nc.dram_tensor("b1", [HID, 1], F32, kind="ExternalInput").ap()
        sel_d = nc.dram_tensor("sel", [128, NCLS, HID], F32,
                               kind="ExternalInput").ap()
        id_d = nc.dram_tensor("ident", [HID, HID], F16,
                              kind="ExternalInput").ap()
        o_d = nc.dram_tensor("t2", [nloc, HID], F16,
                             kind="ExternalOutput").ap()
        ov = o_d.rearrange("(w p) f -> p w f", p=128)
    else:
        b_d = nc.dram_tensor("b2", [DOUT, 1], F32, kind="ExternalInput").ap()
        sel_d = nc.dram_tensor("selw2", [128, NCLS, DOUT], F32,
                               kind="ExternalInput").ap()
        o_d = nc.dram_tensor("o2", [DOUT, nloc], F32,
                             kind="ExternalOutput").ap()
        ov = o_d

    with tile.TileContext(nc) as tc:
        with (
            tc.tile_pool(name="cst", bufs=1) as cst,
            tc.tile_pool(name="ip", bufs=3) as ip,
            tc.tile_pool(name="vp", bufs=2) as vp,
            tc.tile_pool(name="pp", bufs=3) as pp,
            tc.tile_pool(name="sm", bufs=3) as sm,
            tc.tile_pool(name="pY", bufs=2, space="PSUM") as pY,
            tc.tile_pool(name="pT", bufs=2, space="PSUM") as pT,
            tc.tile_pool(name="stg", bufs=2) as stg,
        ):
            nc.gpsimd.load_library(library_config.mlp)
            tabt = cst.tile([128, (nwt + 1) * 128], F16)
            nc.sync.dma_start(out=tabt[:], in_=tab_d[:])
            dvw = cst.tile([64, nloc], F16)
            nc.sync.dma_start(out=dvw[:], in_=dinvw_d[:])
            nb_ = HID if layer == 1 else DOUT
            bt = cst.tile([nb_, 1], F32)
            nc.sync.dma_start(out=bt[:], in_=b_d[:])
            selt = cst.tile([128, NCLS, nb_], F32)
            nc.sync.dma_start(out=selt[:], in_=sel_d[:])
            if layer == 1:
                idt = cst.tile([HID, HID], F16)
                nc.sync.dma_start(out=idt[:], in_=id_d[:])

            it_max = cg_max // 16
            for g in range(ngrp):
                cg = int(Cg[g])
                t0 = int(grpbase[g])
                it = ip.tile([128, it_max], I16, tag="idx")
                nc.sync.dma_start(out=it[:, :cg // 16],
                                  in_=idx_d[:, t0 // 16:(t0 + cg) // 16])
                va = vp.tile([128, 1, cg_max], F16, tag="va")
                # <=4096-idx gathers fit the ~256-desc/engine SWDGE ring, so
                # gather N+1's descriptor generation overlaps gather N's drain
                nch = (cg + 4095) // 4096
                bnds = [(cg * i // nch) // 128 * 128 for i in range(nch + 1)]
                bnds[-1] = cg
                for c0, c1 in zip(bnds[:-1], bnds[1:]):
                    _sbuf_gather(nc, va[:, :, c0:c1], tabt,
                                 it[:, c0 // 16:c1 // 16], c1 - c0, 0)
                Y = pY.tile([nb_, WB], F32)
                for s in range(NCLS):
                    Kgs = int(K[g, s])
                    Ps = pp.tile([128, WB], F32, tag=f"P{s}")
                    _reduce_cls(nc, Ps, va, int(offs[g, s]), Kgs)
                    nc.tensor.matmul(out=Y[:], lhsT=selt[:, s, :], rhs=Ps[:],
                                     start=(s == 0), stop=(s == NCLS - 1))
                dsl = dvw[0:nb_, g * WB:(g + 1) * WB]
                if layer == 1:
                    yd = sm.tile([HID, WB], F32, tag="yd")
                    nc.vector.scalar_tensor_tensor(
                        out=yd[:], in0=Y[:], scalar=1.0, in1=dsl,
                        op0=mybir.AluOpType.mult, op1=mybir.AluOpType.mult,
                    )
                    r = sm.tile([HID, WB], F32, tag="r")
                    nc.scalar.activation(
                        out=r[:], in_=yd[:],
                        func=mybir.ActivationFunctionType.Relu,
                        bias=bt[:, 0:1],
                    )
                    t2c = sm.tile([HID, WB], F16, tag="t2c")
                    nc.vector.scalar_tensor_tensor(
                        out=t2c[:], in0=r[:], scalar=1.0, in1=dsl,
                        op0=mybir.AluOpType.mult, op1=mybir.AluOpType.mult,
                    )
                    stage = stg.tile([128, GW, HID], F16)
                    for j in range(GW):
                        tp = pT.tile([128, HID], F16)
                        nc.tensor.transpose(
                            out=tp[:], in_=t2c[:, j * 128:(j + 1) * 128],
                            identity=idt[:])
                        nc.vector.tensor_copy(out=stage[:, j, :], in_=tp[:])
                    nc.sync.dma_start(out=ov[:, g * GW:(g + 1) * GW, :],
                                      in_=stage[:])
                else:
                    od = sm.tile([DOUT, WB], F32, tag="od")
                    nc.vector.scalar_tensor_tensor(
                        out=od[:], in0=Y[:], scalar=1.0, in1=dsl,
                        op0=mybir.AluOpType.mult, op1=mybir.AluOpType.mult,
                    )
                    stage = stg.tile([DOUT, WB], F32)
                    nc.vector.tensor_scalar_add(
                        out=stage[:], in0=od[:], scalar1=bt[:, 0:1],
                    )
                    nc.sync.dma_start(out=ov[:, g * WB:(g + 1) * WB],
                                      in_=stage[:])
    nc.compile()
    return nc


# ----------------------------------------------------------------------------
# driver
# ----------------------------------------------------------------------------

_PROG_CACHE = {}


def _run(nc, in_maps):
    trace = os.environ.get("GCN_TRACE", "0") == "1"
    res = bass_utils.run_bass_kernel_spmd(
        nc, in_maps, core_ids=list(range(NDEV)), trace=trace)
    if res.exec_time_ns is not None:
        LAST_EXEC_NS.append(int(res.exec_time_ns))
    return res.results


def _build_table(plan, t_node):
    nwt = plan["nwt"]
    tab = np.zeros((128, nwt + 1, NCLS, HID), np.float16)
    q, s, u = plan["slot_q"], plan["slot_s"], plan["slot_u"]
    tab[q % 128, q // 128, s] = t_node[u]
    return np.ascontiguousarray(tab.reshape(128, (nwt + 1) * 128))


def kernel(x, edge_index, W1, b1, W2, b2):
    LAST_EXEC_NS.clear()
    x = np.asarray(x, np.float32)
    W1 = np.asarray(W1, np.float32)
    b1 = np.asarray(b1, np.float32)
    W2 = np.asarray(W2, np.float32)
    b2 = np.asarray(b2, np.float32)

    plan = _plan(np.asarray(edge_index))
    key = ("v3", plan["nwin"], plan["T_dev"], int(plan["K"].sum()))
    if key not in _PROG_CACHE:
        _PROG_CACHE.clear()
        _PROG_CACHE[key] = (_build_A(plan), _build_BC(plan, 1),
                            _build_BC(plan, 2))
    ncA, ncB, ncC = _PROG_CACHE[key]

    ridx, nor = plan["ridx"], plan["node_of_rank"]
    npad, nloc = plan["npad"], plan["nloc"]

    xfull = np.zeros((npad, DIN), np.float32)
    xfull[plan["rank_n"]] = x
    w1r = np.ascontiguousarray(
        W1.reshape(2, 128, HID).transpose(1, 0, 2))
    inA = [{"xT": np.ascontiguousarray(xfull[ridx[d]].T),
            "dinva": plan["dinva"][d], "w1": w1r} for d in range(NDEV)]
    resA = _run(ncA, inA)
    t1n = np.zeros((N, HID), np.float16)
    for d in range(NDEV):
        m = nor[ridx[d]] >= 0
        t1n[nor[ridx[d]][m]] = resA[d]["t1"][m]

    sel = np.zeros((128, NCLS, HID), np.float32)
    for s in range(NCLS):
        sel[16 * s + np.arange(HID), s, np.arange(HID)] = 1.0
    inB = [{"tab": _build_table(plan, t1n), "idx": plan["idxw"][d],
            "dinvw": plan["dinvw"][d], "b1": b1[:, None].astype(np.float32),
            "sel": sel, "ident": np.eye(HID, dtype=np.float16)}
           for d in range(NDEV)]
    resB = _run(ncB, inB)
    t2n = np.zeros((N, HID), np.float16)
    for d in range(NDEV):
        m = nor[ridx[d]] >= 0
        t2n[nor[ridx[d]][m]] = resB[d]["t2"][m]

    selw2 = np.zeros((128, NCLS, DOUT), np.float32)
    for s in range(NCLS):
        selw2[16 * s + np.arange(HID), s, :] = W2
    inC = [{"tab": _build_table(plan, t2n), "idx": plan["idxw"][d],
            "dinvw": plan["dinvw"][d], "b2": b2[:, None].astype(np.float32),
            "selw2": selw2} for d in range(NDEV)]
    resC = _run(ncC, inC)
    out = np.zeros((N, DOUT), np.float32)
    for d in range(NDEV):
        m = nor[ridx[d]] >= 0
        out[nor[ridx[d]][m]] = resC[d]["o2"].T[m]
    return out
